# revision 2
# baseline (speedup 1.0000x reference)
"""Trainium2 Bass kernel for GQA attention (nn_Attention_12197707121071).

Strategy: tensor-parallel across heads over 8 NeuronCores.
  - Each core owns 2 query heads + the 1 KV head serving them (KV heads are
    replicated across core pairs since 4 kv-heads don't divide 8 cores).
  - Host pre-transposes x -> xT (feat, rows) and pre-casts all inputs to
    bf16; wq/wk columns are permuted per-head to [even dims | odd dims] so
    RoPE becomes an operation on partition halves; the 1/sqrt(head_dim)
    score scale is folded into wq/wk on the host.
  - On device: qkv projection (feat-on-partition layout), RoPE fused into
    the PSUM evacuation, causal attention computed transposed (S^T =
    K @ Q^T per block) so the PV matmul needs no transposes and the
    softmax denominator comes from a ones-matmul; softmax max-subtraction
    is skipped (scores ~ N(0,1), exp is safe in fp32).
  - AllGather (one per head, bf16) of the normalized attention outputs,
    then each core computes its 256 output columns of the wo projection.
  - Host gathers per-core (256, 4096) transposed outputs into the full
    (2, 2048, 2048) fp32 output.
"""

import sys
import numpy as np

for _p in (
    "/root/.axon_site",
    "/root/.axon_site/_ro/trn_rl_repo",
    "/root/.axon_site/_ro/pypackages",
    "/opt/trn_rl_repo",
):
    if _p not in sys.path:
        sys.path.append(_p)

import ml_dtypes

BF16 = ml_dtypes.bfloat16

B, S, DIM = 2, 2048, 2048
N_HEADS = 16
N_KV_HEADS = 4
HEAD_DIM = 128
N_CORES = 8
PE_N = 512  # moving-operand free dim per matmul


# --------------------------------------------------------------------------
# device kernel builder
# --------------------------------------------------------------------------

def build_nc(b=B, s=S):
    """Build + compile the SPMD Bass graph (identical on all 8 cores)."""
    from contextlib import ExitStack

    from concourse import bacc, mybir
    import concourse.tile as tile

    dt = mybir.dt
    f32, bf16 = dt.float32, dt.bfloat16
    rows = b * s
    KC = DIM // 128          # contraction chunks (16)
    KHALF = KC // 2
    RCB = s // PE_N          # row chunks per batch
    NKB = s // 128           # key blocks per batch
    NF = DIM // 128          # feature chunks for out-proj (16)

    nc = bacc.Bacc("TRN2", target_bir_lowering=False, debug=False,
                   num_devices=N_CORES)

    d = {}
    d["xT"] = nc.dram_tensor("xT", [DIM, rows], bf16, kind="ExternalInput")
    d["wq"] = nc.dram_tensor("wq", [DIM, 256], bf16, kind="ExternalInput")
    d["wk"] = nc.dram_tensor("wk", [DIM, 128], bf16, kind="ExternalInput")
    d["wv"] = nc.dram_tensor("wv", [DIM, 128], bf16, kind="ExternalInput")
    d["wo"] = nc.dram_tensor("wo", [DIM, 256], bf16, kind="ExternalInput")
    d["cosT"] = nc.dram_tensor("cosT", [64, s], bf16, kind="ExternalInput")
    d["sinT"] = nc.dram_tensor("sinT", [64, s], bf16, kind="ExternalInput")
    d["masks"] = nc.dram_tensor("masks", [4, 128, PE_N], bf16, kind="ExternalInput")
    d["onesw"] = nc.dram_tensor("onesw", [128, 128], bf16, kind="ExternalInput")
    d["ident"] = nc.dram_tensor("ident", [128, 128], bf16, kind="ExternalInput")
    d["out"] = nc.dram_tensor("out", [256, rows], f32, kind="ExternalOutput")

    bounce = [nc.dram_tensor(f"bounce{h}", [128, rows], bf16) for h in range(2)]
    gath = [nc.dram_tensor(f"gath{h}", [128 * N_CORES, rows], bf16,
                           addr_space="Shared") for h in range(2)]

    Exp = mybir.ActivationFunctionType.Exp

    with tile.TileContext(nc) as tc, ExitStack() as ctx:
        cpool = ctx.enter_context(tc.tile_pool(name="consts", bufs=1))
        apool = ctx.enter_context(tc.tile_pool(name="acts", bufs=1))
        tpool = ctx.enter_context(tc.tile_pool(name="tmps", bufs=4))
        epool = ctx.enter_context(tc.tile_pool(name="exps", bufs=4))
        rpool = ctx.enter_context(tc.tile_pool(name="recip", bufs=2))
        mmps = ctx.enter_context(tc.tile_pool(name="mmps", bufs=4, space="PSUM"))
        stps = ctx.enter_context(tc.tile_pool(name="stps", bufs=2, space="PSUM"))
        otps = ctx.enter_context(tc.tile_pool(name="otps", bufs=1, space="PSUM"))
        dnps = ctx.enter_context(tc.tile_pool(name="dnps", bufs=1, space="PSUM"))

        # ---- constants into SBUF
        wq_sb = cpool.tile([128, KC, 256], bf16, tag="wq")
        nc.sync.dma_start(out=wq_sb[:], in_=d["wq"].ap().rearrange("(kc p) f -> p kc f", p=128))
        wk_sb = cpool.tile([128, KC, 128], bf16, tag="wk")
        nc.sync.dma_start(out=wk_sb[:], in_=d["wk"].ap().rearrange("(kc p) f -> p kc f", p=128))
        wv_sb = cpool.tile([128, KC, 128], bf16, tag="wv")
        nc.sync.dma_start(out=wv_sb[:], in_=d["wv"].ap().rearrange("(kc p) f -> p kc f", p=128))
        wo_sb = cpool.tile([128, NF, 256], bf16, tag="wo")
        nc.sync.dma_start(out=wo_sb[:], in_=d["wo"].ap().rearrange("(kc p) f -> p kc f", p=128))
        cos_sb = cpool.tile([64, s], bf16, tag="cos")
        nc.sync.dma_start(out=cos_sb[:], in_=d["cosT"][:, :])
        sin_sb = cpool.tile([64, s], bf16, tag="sin")
        nc.sync.dma_start(out=sin_sb[:], in_=d["sinT"][:, :])
        msk_sb = cpool.tile([128, 4, PE_N], bf16, tag="msk")
        nc.sync.dma_start(out=msk_sb[:], in_=d["masks"].ap().rearrange("o p q -> p o q"))
        ones_sb = cpool.tile([128, 128], bf16, tag="ones")
        nc.sync.dma_start(out=ones_sb[:], in_=d["onesw"][:, :])
        id_sb = cpool.tile([128, 128], bf16, tag="id")
        nc.sync.dma_start(out=id_sb[:], in_=d["ident"][:, :])

        # ---- persistent activations
        q_sb = apool.tile([128, 2, rows], bf16, tag="q")     # qT per head (rope'd)
        kT_sb = apool.tile([128, rows], bf16, tag="k")       # kT (rope'd)
        vT_sb = apool.tile([128, rows], bf16, tag="vt")      # vT (pre-transpose)
        vn_sb = apool.tile([128, b * NKB, 128], bf16, tag="vn")  # v natural, per key-block
        ot_sb = apool.tile([128, 2, rows], bf16, tag="ot")   # normalized attn out^T

        def qkv_batch(bi, xpool):
            xb = [xpool.tile([128, KHALF, s], bf16, tag="xb", name=f"xb{bi}_{i}") for i in range(2)]
            for half in range(2):
                for kc in range(KHALF):
                    kcg = half * KHALF + kc
                    nc.sync.dma_start(
                        out=xb[half][:, kc, :],
                        in_=d["xT"][kcg * 128:(kcg + 1) * 128, bi * s:(bi + 1) * s])
            # mb: 0 = q head0, 1 = q head1, 2 = k, 3 = v
            for mb in range(4):
                for rcg0 in range(0, RCB, 4):
                    rcs = list(range(rcg0, min(rcg0 + 4, RCB)))
                    psums = [mmps.tile([128, PE_N], f32, tag="mm", name=f"mm{mb}_{rc}") for rc in rcs]
                    for kcg in range(KC):
                        half, kc = divmod(kcg, KHALF)
                        if mb == 0:
                            w_ap = wq_sb[:, kcg, 0:128]
                        elif mb == 1:
                            w_ap = wq_sb[:, kcg, 128:256]
                        elif mb == 2:
                            w_ap = wk_sb[:, kcg, :]
                        else:
                            w_ap = wv_sb[:, kcg, :]
                        for ji, rc in enumerate(rcs):
                            nc.tensor.matmul(
                                psums[ji][:], w_ap,
                                xb[half][:, kc, rc * PE_N:(rc + 1) * PE_N],
                                start=(kcg == 0), stop=(kcg == KC - 1))
                    for ji, rc in enumerate(rcs):
                        cols = bi * s + rc * PE_N
                        scol = rc * PE_N
                        if mb <= 2:
                            dst = (q_sb[:, mb, cols:cols + PE_N] if mb < 2
                                   else kT_sb[:, cols:cols + PE_N])
                            pr = psums[ji][0:64, :]
                            pi = psums[ji][64:128, :]
                            csl = cos_sb[:, scol:scol + PE_N]
                            ssl = sin_sb[:, scol:scol + PE_N]
                            t1 = tpool.tile([64, PE_N], f32, tag="t1")
                            t2 = tpool.tile([64, PE_N], f32, tag="t2")
                            nc.vector.tensor_mul(t1[:], pr, csl)
                            nc.vector.tensor_mul(t2[:], pi, ssl)
                            nc.vector.tensor_sub(dst[0:64, :], t1[:], t2[:])
                            t3 = tpool.tile([64, PE_N], f32, tag="t1")
                            t4 = tpool.tile([64, PE_N], f32, tag="t2")
                            nc.vector.tensor_mul(t3[:], pr, ssl)
                            nc.vector.tensor_mul(t4[:], pi, csl)
                            nc.vector.tensor_add(dst[64:128, :], t3[:], t4[:])
                        else:
                            nc.scalar.copy(vT_sb[:, cols:cols + PE_N], psums[ji][:])
            # transpose v into natural (keys-on-partition) layout
            for kb in range(NKB):
                kbg = bi * NKB + kb
                tt = stps.tile([128, 128], bf16, tag="st")
                nc.tensor.transpose(tt[:], vT_sb[:, kbg * 128:(kbg + 1) * 128], id_sb[:])
                nc.vector.tensor_copy(vn_sb[:, kbg, :], tt[:])

        def attn(bi, h):
            base = bi * s
            for qc in range(RCB):
                nkb = (qc + 1) * (PE_N // 128)
                otp = otps.tile([128, PE_N], f32, tag="ot")
                dnp = dnps.tile([128, PE_N], f32, tag="dn")
                qs = q_sb[:, h, base + qc * PE_N: base + (qc + 1) * PE_N]
                for kb in range(nkb):
                    stp = stps.tile([128, PE_N], f32, tag="st")
                    nc.tensor.matmul(
                        stp[:], kT_sb[:, base + kb * 128: base + (kb + 1) * 128],
                        qs, start=True, stop=True)
                    ex = epool.tile([128, PE_N], bf16, tag="ex")
                    nc.scalar.activation(ex[:], stp[:], Exp)
                    o = kb - 4 * qc
                    if o >= 0:
                        nc.vector.tensor_mul(ex[:], ex[:], msk_sb[:, o, :])
                    nc.tensor.matmul(otp[:], vn_sb[:, bi * NKB + kb, :], ex[:],
                                     start=(kb == 0), stop=(kb == nkb - 1))
                    nc.tensor.matmul(dnp[:], ones_sb[:], ex[:],
                                     start=(kb == 0), stop=(kb == nkb - 1))
                rc_t = rpool.tile([128, PE_N], f32, tag="rc")
                nc.vector.reciprocal(rc_t[:], dnp[:])
                nc.vector.tensor_mul(
                    ot_sb[:, h, base + qc * PE_N: base + (qc + 1) * PE_N],
                    otp[:], rc_t[:])

        def ship_head(h):
            nc.sync.dma_start(out=bounce[h][:, :], in_=ot_sb[:, h, :])
            nc.gpsimd.collective_compute(
                "AllGather", mybir.AluOpType.bypass,
                replica_groups=[list(range(N_CORES))],
                ins=[bounce[h].ap().opt()],
                outs=[gath[h].ap().opt()])

        with ExitStack() as p12:
            xpool = p12.enter_context(tc.tile_pool(name="xb", bufs=2))
            qkv_batch(0, xpool)
            if b > 1:
                attn(0, 0)
                attn(0, 1)
                qkv_batch(1, xpool)
                attn(1, 0)
                ship_head(0)
                attn(1, 1)
                ship_head(1)
            else:
                attn(0, 0)
                ship_head(0)
                attn(0, 1)
                ship_head(1)

        with ExitStack() as p3:
            gpool = p3.enter_context(tc.tile_pool(name="gt", bufs=18))
            ospool = p3.enter_context(tc.tile_pool(name="os", bufs=4))
            NRC = rows // PE_N
            for j0 in range(0, NRC, 2):
                jj = list(range(j0, min(j0 + 2, NRC)))
                nj = len(jj)
                gts = []
                for f in range(NF):
                    gt = gpool.tile([128, nj * PE_N], bf16, tag="g")
                    src = gath[f // 8]
                    fl = f % 8
                    nc.sync.dma_start(
                        out=gt[:],
                        in_=src[fl * 128:(fl + 1) * 128, j0 * PE_N:(j0 + nj) * PE_N])
                    gts.append(gt)
                psums = [[mmps.tile([128, PE_N], f32, tag="mm", name=f"po{c}_{j}") for j in jj]
                         for c in range(2)]
                for c in range(2):
                    for f in range(NF):
                        for ji in range(nj):
                            nc.tensor.matmul(
                                psums[c][ji][:], wo_sb[:, f, c * 128:(c + 1) * 128],
                                gts[f][:, ji * PE_N:(ji + 1) * PE_N],
                                start=(f == 0), stop=(f == NF - 1))
                for c in range(2):
                    for ji, j in enumerate(jj):
                        osb = ospool.tile([128, PE_N], f32, tag="os")
                        nc.scalar.copy(osb[:], psums[c][ji][:])
                        nc.sync.dma_start(
                            out=d["out"][c * 128:(c + 1) * 128,
                                         j * PE_N:(j + 1) * PE_N],
                            in_=osb[:])

    nc.compile()
    return nc


# --------------------------------------------------------------------------
# host-side input prep / output assembly
# --------------------------------------------------------------------------

def prep_in_maps(x, freqs_cos, freqs_sin, wq, wk, wv, wo, b=B, s=S):
    """Shard + preprocess full fp32 inputs into 8 per-core input dicts."""
    n_heads = N_HEADS
    n_kv = N_KV_HEADS
    rows = b * s
    x = np.asarray(x, np.float32)
    xT = np.ascontiguousarray(x.reshape(rows, DIM).T).astype(BF16)

    sc = float(HEAD_DIM) ** -0.25
    perm64 = np.concatenate([np.arange(0, 128, 2), np.arange(1, 128, 2)])
    qcols = np.concatenate([h * 128 + perm64 for h in range(n_heads)])
    kcols = np.concatenate([g * 128 + perm64 for g in range(n_kv)])
    wq_p = (np.asarray(wq, np.float32) * sc)[:, qcols].astype(BF16)
    wk_p = (np.asarray(wk, np.float32) * sc)[:, kcols].astype(BF16)
    wv_p = np.asarray(wv, np.float32).astype(BF16)
    # wo rows in "gathered" order: AG0 = even heads (core order), AG1 = odd
    head_order = [2 * i for i in range(N_CORES)] + [2 * i + 1 for i in range(N_CORES)]
    wo_p = np.concatenate(
        [np.asarray(wo, np.float32)[h * 128:(h + 1) * 128, :] for h in head_order],
        axis=0).astype(BF16)

    cosT = np.ascontiguousarray(np.asarray(freqs_cos, np.float32).T).astype(BF16)
    sinT = np.ascontiguousarray(np.asarray(freqs_sin, np.float32).T).astype(BF16)

    qi = np.arange(PE_N)[None, :]
    ki = np.arange(128)[:, None]
    masks = np.stack([(qi - o * 128 >= ki) for o in range(4)]).astype(BF16)
    onesw = np.ones((128, 128), BF16)
    ident = np.eye(128, dtype=BF16)

    in_maps = []
    for c in range(N_CORES):
        g = c // 2
        in_maps.append({
            "xT": xT,
            "wq": np.ascontiguousarray(wq_p[:, c * 256:(c + 1) * 256]),
            "wk": np.ascontiguousarray(wk_p[:, g * 128:(g + 1) * 128]),
            "wv": np.ascontiguousarray(wv_p[:, g * 128:(g + 1) * 128]),
            "wo": np.ascontiguousarray(wo_p[:, c * 256:(c + 1) * 256]),
            "cosT": cosT,
            "sinT": sinT,
            "masks": masks,
            "onesw": onesw,
            "ident": ident,
        })
    return in_maps


def assemble_output(results, b=B, s=S):
    rows = b * s
    out = np.empty((rows, DIM), np.float32)
    for c in range(N_CORES):
        out[:, c * 256:(c + 1) * 256] = results[c]["out"].T
    return out.reshape(b, s, DIM)


_NC_CACHE = {}


def _get_nc(b=B, s=S):
    key = (b, s)
    if key not in _NC_CACHE:
        _NC_CACHE[key] = build_nc(b, s)
    return _NC_CACHE[key]


def run(inputs, trace=False, b=B, s=S):
    """Run the kernel; returns (output, BassKernelResults)."""
    from concourse import bass_utils
    nc = _get_nc(b, s)
    in_maps = prep_in_maps(**inputs, b=b, s=s)
    res = bass_utils.run_bass_kernel_spmd(
        nc, in_maps, core_ids=list(range(N_CORES)), trace=trace)
    return assemble_output(res.results, b, s), res


def kernel(**inputs):
    out, _ = run(inputs)
    return out


# revision 4
# speedup vs baseline: 1.2901x; 1.2901x over previous
"""Trainium2 Bass kernel for GQA attention (nn_Attention_12197707121071).

Strategy: tensor-parallel across heads over 8 NeuronCores.
  - Each core owns 2 query heads + the 1 KV head serving them (KV heads are
    replicated across core pairs since 4 kv-heads don't divide 8 cores).
  - Host pre-transposes x -> xT (feat, rows) and pre-casts all inputs to
    bf16; wq/wk columns are permuted per-head to [even dims | odd dims] so
    RoPE becomes an operation on partition halves; the 1/sqrt(head_dim)
    score scale is folded into wq/wk on the host; the RoPE rotation sign is
    folded into the sin table ([-sin; +sin] over the two partition halves).
  - On device: qkv projection (feat-on-partition layout), RoPE fused into
    the PSUM evacuation (ACT casts PSUM->bf16, DVE does swap + 3 bf16 ops),
    causal attention computed transposed (S^T = K @ Q^T per block) so the
    PV matmul needs no transposes and the softmax denominator comes from a
    ones-matmul; softmax max-subtraction is skipped (scores ~ N(0,1));
    the attention inner loop is software-pipelined (S^T matmul+exp issued
    2 blocks ahead of the PV/denominator matmuls).
  - AllGather per (head, batch) of normalized attention outputs (4 x 0.5MB
    bf16), overlapped with remaining attention compute; each core then
    computes its 256 output columns of the wo projection.
  - Host gathers per-core (256, 4096) transposed outputs into the full
    (2, 2048, 2048) fp32 output.
"""

import sys
import numpy as np

for _p in (
    "/root/.axon_site",
    "/root/.axon_site/_ro/trn_rl_repo",
    "/root/.axon_site/_ro/pypackages",
    "/opt/trn_rl_repo",
):
    if _p not in sys.path:
        sys.path.append(_p)

import ml_dtypes

BF16 = ml_dtypes.bfloat16

B, S, DIM = 2, 2048, 2048
N_HEADS = 16
N_KV_HEADS = 4
HEAD_DIM = 128
N_CORES = 8
PE_N = 512  # moving-operand free dim per matmul


# --------------------------------------------------------------------------
# device kernel builder
# --------------------------------------------------------------------------

def build_nc(b=B, s=S):
    """Build + compile the SPMD Bass graph (identical on all 8 cores)."""
    from contextlib import ExitStack

    from concourse import bacc, mybir
    import concourse.tile as tile

    dt = mybir.dt
    f32, bf16 = dt.float32, dt.bfloat16
    rows = b * s
    KC = DIM // 128          # contraction chunks (16)
    KHALF = KC // 2
    RCB = s // PE_N          # row chunks per batch
    NKB = s // 128           # key blocks per batch
    NF = DIM // 128          # feature chunks for out-proj (16)

    nc = bacc.Bacc("TRN2", target_bir_lowering=False, debug=False,
                   num_devices=N_CORES)

    d = {}
    d["xT"] = nc.dram_tensor("xT", [DIM, rows], bf16, kind="ExternalInput")
    d["wq"] = nc.dram_tensor("wq", [DIM, 256], bf16, kind="ExternalInput")
    d["wk"] = nc.dram_tensor("wk", [DIM, 128], bf16, kind="ExternalInput")
    d["wv"] = nc.dram_tensor("wv", [DIM, 128], bf16, kind="ExternalInput")
    d["wo"] = nc.dram_tensor("wo", [DIM, 256], bf16, kind="ExternalInput")
    d["cosF"] = nc.dram_tensor("cosF", [128, s], bf16, kind="ExternalInput")
    d["sinPM"] = nc.dram_tensor("sinPM", [128, s], bf16, kind="ExternalInput")
    d["masks"] = nc.dram_tensor("masks", [4, 128, PE_N], bf16, kind="ExternalInput")
    d["onesw"] = nc.dram_tensor("onesw", [128, 128], bf16, kind="ExternalInput")
    d["ident"] = nc.dram_tensor("ident", [128, 128], bf16, kind="ExternalInput")
    d["out"] = nc.dram_tensor("out", [256, rows], f32, kind="ExternalOutput")

    bounce = [[nc.dram_tensor(f"bounce{h}_{bi}", [128, s], bf16)
               for bi in range(b)] for h in range(2)]
    gath = [[nc.dram_tensor(f"gath{h}_{bi}", [128 * N_CORES, s], bf16,
                            addr_space="Shared")
             for bi in range(b)] for h in range(2)]

    Exp = mybir.ActivationFunctionType.Exp

    with tile.TileContext(nc) as tc, ExitStack() as ctx:
        cpool = ctx.enter_context(tc.tile_pool(name="consts", bufs=1))
        apool = ctx.enter_context(tc.tile_pool(name="acts", bufs=1))
        tpool = ctx.enter_context(tc.tile_pool(name="tmps", bufs=4))
        epool = ctx.enter_context(tc.tile_pool(name="exps", bufs=6))
        rpool = ctx.enter_context(tc.tile_pool(name="recip", bufs=2))
        mmps = ctx.enter_context(tc.tile_pool(name="mmps", bufs=2, space="PSUM"))
        stps = ctx.enter_context(tc.tile_pool(name="stps", bufs=4, space="PSUM"))
        otps = ctx.enter_context(tc.tile_pool(name="otps", bufs=1, space="PSUM"))
        dnps = ctx.enter_context(tc.tile_pool(name="dnps", bufs=1, space="PSUM"))

        # ---- constants into SBUF
        wq_sb = cpool.tile([128, KC, 256], bf16, tag="wq")
        nc.sync.dma_start(out=wq_sb[:], in_=d["wq"].ap().rearrange("(kc p) f -> p kc f", p=128))
        wk_sb = cpool.tile([128, KC, 128], bf16, tag="wk")
        nc.sync.dma_start(out=wk_sb[:], in_=d["wk"].ap().rearrange("(kc p) f -> p kc f", p=128))
        wv_sb = cpool.tile([128, KC, 128], bf16, tag="wv")
        nc.sync.dma_start(out=wv_sb[:], in_=d["wv"].ap().rearrange("(kc p) f -> p kc f", p=128))
        wo_sb = cpool.tile([128, NF, 256], bf16, tag="wo")
        nc.sync.dma_start(out=wo_sb[:], in_=d["wo"].ap().rearrange("(kc p) f -> p kc f", p=128))
        cos_sb = cpool.tile([128, s], bf16, tag="cos")
        nc.sync.dma_start(out=cos_sb[:], in_=d["cosF"][:, :])
        sin_sb = cpool.tile([128, s], bf16, tag="sin")
        nc.sync.dma_start(out=sin_sb[:], in_=d["sinPM"][:, :])
        msk_sb = cpool.tile([128, 4, PE_N], bf16, tag="msk")
        nc.sync.dma_start(out=msk_sb[:], in_=d["masks"].ap().rearrange("o p q -> p o q"))
        ones_sb = cpool.tile([128, 128], bf16, tag="ones")
        nc.sync.dma_start(out=ones_sb[:], in_=d["onesw"][:, :])
        id_sb = cpool.tile([128, 128], bf16, tag="id")
        nc.sync.dma_start(out=id_sb[:], in_=d["ident"][:, :])

        # ---- persistent activations
        q_sb = apool.tile([128, 2, rows], bf16, tag="q")     # qT per head (rope'd)
        kT_sb = apool.tile([128, rows], bf16, tag="k")       # kT (rope'd)
        vT_sb = apool.tile([128, rows], bf16, tag="vt")      # vT (pre-transpose)
        vn_sb = apool.tile([128, b * NKB, 128], bf16, tag="vn")  # v natural, per key-block
        ot_sb = apool.tile([128, 2, rows], bf16, tag="ot")   # normalized attn out^T

        def rope_evac(psum, dst, scol):
            """dst = rope(psum) in bf16: c1 = cast(psum); sw = swap halves;
            dst = c1*cosF + sw*sinPM (sign folded into sinPM)."""
            c1 = tpool.tile([128, PE_N], bf16, tag="c1", name="c1")
            nc.scalar.copy(c1[:], psum[:])
            sw = tpool.tile([128, PE_N], bf16, tag="sw", name="sw")
            nc.vector.tensor_copy(sw[0:64, :], c1[64:128, :])
            nc.vector.tensor_copy(sw[64:128, :], c1[0:64, :])
            m1 = tpool.tile([128, PE_N], bf16, tag="m1", name="m1")
            nc.vector.tensor_mul(m1[:], c1[:], cos_sb[:, scol:scol + PE_N])
            nc.vector.tensor_mul(sw[:], sw[:], sin_sb[:, scol:scol + PE_N])
            nc.vector.tensor_add(dst, m1[:], sw[:])

        def qkv_batch(bi, xpool):
            xb = [xpool.tile([128, KHALF, s], bf16, tag="xb", name=f"xb{bi}_{i}")
                  for i in range(2)]
            for half in range(2):
                for kc in range(KHALF):
                    kcg = half * KHALF + kc
                    nc.sync.dma_start(
                        out=xb[half][:, kc, :],
                        in_=d["xT"][kcg * 128:(kcg + 1) * 128, bi * s:(bi + 1) * s])
            # mb: 0 = q head0, 1 = q head1, 2 = k, 3 = v
            for mb in range(4):
                for rcg0 in range(0, RCB, 2):
                    rcs = list(range(rcg0, min(rcg0 + 2, RCB)))
                    psums = [mmps.tile([128, PE_N], f32, tag="mm", name=f"mm{mb}_{rc}")
                             for rc in rcs]
                    for kcg in range(KC):
                        half, kc = divmod(kcg, KHALF)
                        if mb == 0:
                            w_ap = wq_sb[:, kcg, 0:128]
                        elif mb == 1:
                            w_ap = wq_sb[:, kcg, 128:256]
                        elif mb == 2:
                            w_ap = wk_sb[:, kcg, :]
                        else:
                            w_ap = wv_sb[:, kcg, :]
                        for ji, rc in enumerate(rcs):
                            nc.tensor.matmul(
                                psums[ji][:], w_ap,
                                xb[half][:, kc, rc * PE_N:(rc + 1) * PE_N],
                                start=(kcg == 0), stop=(kcg == KC - 1))
                    for ji, rc in enumerate(rcs):
                        cols = bi * s + rc * PE_N
                        scol = rc * PE_N
                        if mb <= 2:
                            dst = (q_sb[:, mb, cols:cols + PE_N] if mb < 2
                                   else kT_sb[:, cols:cols + PE_N])
                            rope_evac(psums[ji], dst, scol)
                        else:
                            nc.scalar.copy(vT_sb[:, cols:cols + PE_N], psums[ji][:])
            # transpose v into natural (keys-on-partition) layout
            for kb in range(NKB):
                kbg = bi * NKB + kb
                tt = stps.tile([128, 128], bf16, tag="st", name=f"tt{kbg}")
                nc.tensor.transpose(tt[:], vT_sb[:, kbg * 128:(kbg + 1) * 128], id_sb[:])
                nc.vector.tensor_copy(vn_sb[:, kbg, :], tt[:])

        def attn(bi, h):
            base = bi * s
            for qc in range(RCB):
                nkb = (qc + 1) * (PE_N // 128)
                otp = otps.tile([128, PE_N], f32, tag="ot", name="otp")
                dnp = dnps.tile([128, PE_N], f32, tag="dn", name="dnp")
                qs = q_sb[:, h, base + qc * PE_N: base + (qc + 1) * PE_N]
                exps = {}

                def issue_st(kb):
                    stp = stps.tile([128, PE_N], f32, tag="st", name=f"st{kb}")
                    nc.tensor.matmul(
                        stp[:], kT_sb[:, base + kb * 128: base + (kb + 1) * 128],
                        qs, start=True, stop=True)
                    ex = epool.tile([128, PE_N], bf16, tag="ex", name=f"ex{kb}")
                    nc.scalar.activation(ex[:], stp[:], Exp)
                    o = kb - 4 * qc
                    if o >= 0:
                        nc.vector.tensor_mul(ex[:], ex[:], msk_sb[:, o, :])
                    exps[kb] = ex

                def issue_pv(kb):
                    ex = exps.pop(kb)
                    nc.tensor.matmul(otp[:], vn_sb[:, bi * NKB + kb, :], ex[:],
                                     start=(kb == 0), stop=(kb == nkb - 1))
                    nc.tensor.matmul(dnp[:], ones_sb[:], ex[:],
                                     start=(kb == 0), stop=(kb == nkb - 1))

                DEPTH = 2
                for kb in range(nkb):
                    issue_st(kb)
                    if kb >= DEPTH:
                        issue_pv(kb - DEPTH)
                for kb in range(max(0, nkb - DEPTH), nkb):
                    issue_pv(kb)

                rc_t = rpool.tile([128, PE_N], f32, tag="rc", name="rc_t")
                nc.vector.reciprocal_approx_fast(out=rc_t[:], in_=dnp[:])
                nc.vector.tensor_mul(
                    ot_sb[:, h, base + qc * PE_N: base + (qc + 1) * PE_N],
                    otp[:], rc_t[:])

        def ship(h, bi):
            nc.sync.dma_start(out=bounce[h][bi][:, :],
                              in_=ot_sb[:, h, bi * s:(bi + 1) * s])
            nc.gpsimd.collective_compute(
                "AllGather", mybir.AluOpType.bypass,
                replica_groups=[list(range(N_CORES))],
                ins=[bounce[h][bi].ap().opt()],
                outs=[gath[h][bi].ap().opt()])

        def outproj_batch(bi, gpool, ospool):
            # out^T[c*128:(c+1)*128, rows of batch bi] = sum_f wo[f,c] @ gath
            for j0 in range(0, RCB, 2):
                jj = list(range(j0, min(j0 + 2, RCB)))
                nj = len(jj)
                gts = []
                for f in range(NF):
                    gt = gpool.tile([128, nj * PE_N], bf16, tag="g", name=f"g{f}")
                    src = gath[f // 8][bi]
                    fl = f % 8
                    nc.sync.dma_start(
                        out=gt[:],
                        in_=src[fl * 128:(fl + 1) * 128,
                                j0 * PE_N:(j0 + nj) * PE_N])
                    gts.append(gt)
                # 4 concurrent accumulators (2 col-chunks x 2 row-chunks)
                psums = [[stps.tile([128, PE_N], f32, tag="st", name=f"po{c}_{j}")
                          for j in jj] for c in range(2)]
                for f in range(NF):
                    for c in range(2):
                        for ji in range(nj):
                            nc.tensor.matmul(
                                psums[c][ji][:], wo_sb[:, f, c * 128:(c + 1) * 128],
                                gts[f][:, ji * PE_N:(ji + 1) * PE_N],
                                start=(f == 0), stop=(f == NF - 1))
                for c in range(2):
                    for ji, j in enumerate(jj):
                        osb = ospool.tile([128, PE_N], f32, tag="os", name="osb")
                        nc.scalar.copy(osb[:], psums[c][ji][:])
                        nc.sync.dma_start(
                            out=d["out"][c * 128:(c + 1) * 128,
                                         bi * s + j * PE_N:
                                         bi * s + (j + 1) * PE_N],
                            in_=osb[:])

        with ExitStack() as p12:
            xpool = p12.enter_context(tc.tile_pool(name="xb", bufs=2))
            qkv_batch(0, xpool)
            attn(0, 0)
            ship(0, 0)
            attn(0, 1)
            ship(1, 0)
            if b > 1:
                qkv_batch(1, xpool)
                attn(1, 0)
                ship(0, 1)
                attn(1, 1)
                ship(1, 1)

        with ExitStack() as p3:
            gpool = p3.enter_context(tc.tile_pool(name="gt", bufs=18))
            ospool = p3.enter_context(tc.tile_pool(name="os", bufs=4))
            for bi in range(b):
                outproj_batch(bi, gpool, ospool)

    nc.compile()
    return nc


# --------------------------------------------------------------------------
# host-side input prep / output assembly
# --------------------------------------------------------------------------

def prep_in_maps(x, freqs_cos, freqs_sin, wq, wk, wv, wo, b=B, s=S):
    """Shard + preprocess full fp32 inputs into 8 per-core input dicts."""
    n_heads = N_HEADS
    n_kv = N_KV_HEADS
    rows = b * s
    x = np.asarray(x, np.float32)
    xT = np.ascontiguousarray(x.reshape(rows, DIM).T).astype(BF16)

    sc = float(HEAD_DIM) ** -0.25
    perm64 = np.concatenate([np.arange(0, 128, 2), np.arange(1, 128, 2)])
    qcols = np.concatenate([h * 128 + perm64 for h in range(n_heads)])
    kcols = np.concatenate([g * 128 + perm64 for g in range(n_kv)])
    wq_p = (np.asarray(wq, np.float32) * sc)[:, qcols].astype(BF16)
    wk_p = (np.asarray(wk, np.float32) * sc)[:, kcols].astype(BF16)
    wv_p = np.asarray(wv, np.float32).astype(BF16)
    # wo rows in "gathered" order: AG0 = even heads (core order), AG1 = odd
    head_order = [2 * i for i in range(N_CORES)] + [2 * i + 1 for i in range(N_CORES)]
    wo_p = np.concatenate(
        [np.asarray(wo, np.float32)[h * 128:(h + 1) * 128, :] for h in head_order],
        axis=0).astype(BF16)

    cosT = np.asarray(freqs_cos, np.float32).T  # (64, s)
    sinT = np.asarray(freqs_sin, np.float32).T
    cosF = np.ascontiguousarray(np.concatenate([cosT, cosT], axis=0)).astype(BF16)
    sinPM = np.ascontiguousarray(np.concatenate([-sinT, sinT], axis=0)).astype(BF16)

    qi = np.arange(PE_N)[None, :]
    ki = np.arange(128)[:, None]
    masks = np.stack([(qi - o * 128 >= ki) for o in range(4)]).astype(BF16)
    onesw = np.ones((128, 128), BF16)
    ident = np.eye(128, dtype=BF16)

    in_maps = []
    for c in range(N_CORES):
        g = c // 2
        in_maps.append({
            "xT": xT,
            "wq": np.ascontiguousarray(wq_p[:, c * 256:(c + 1) * 256]),
            "wk": np.ascontiguousarray(wk_p[:, g * 128:(g + 1) * 128]),
            "wv": np.ascontiguousarray(wv_p[:, g * 128:(g + 1) * 128]),
            "wo": np.ascontiguousarray(wo_p[:, c * 256:(c + 1) * 256]),
            "cosF": cosF,
            "sinPM": sinPM,
            "masks": masks,
            "onesw": onesw,
            "ident": ident,
        })
    return in_maps


def assemble_output(results, b=B, s=S):
    rows = b * s
    out = np.empty((rows, DIM), np.float32)
    for c in range(N_CORES):
        out[:, c * 256:(c + 1) * 256] = results[c]["out"].T
    return out.reshape(b, s, DIM)


_NC_CACHE = {}


def _get_nc(b=B, s=S):
    key = (b, s)
    if key not in _NC_CACHE:
        _NC_CACHE[key] = build_nc(b, s)
    return _NC_CACHE[key]


def run(inputs, trace=False, b=B, s=S):
    """Run the kernel; returns (output, BassKernelResults)."""
    from concourse import bass_utils
    nc = _get_nc(b, s)
    in_maps = prep_in_maps(**inputs, b=b, s=s)
    res = bass_utils.run_bass_kernel_spmd(
        nc, in_maps, core_ids=list(range(N_CORES)), trace=trace)
    return assemble_output(res.results, b, s), res


def kernel(**inputs):
    out, _ = run(inputs)
    return out


# revision 7
# speedup vs baseline: 1.3664x; 1.0591x over previous
"""Trainium2 Bass kernel for GQA attention (nn_Attention_12197707121071).

Strategy: tensor-parallel across heads over 8 NeuronCores.
  - Each core owns 2 query heads + the 1 KV head serving them (KV heads are
    replicated across core pairs since 4 kv-heads don't divide 8 cores).
  - Host pre-transposes x -> xT (feat, rows) and pre-casts all inputs to
    bf16; wq/wk columns are permuted per-head to [even dims | odd dims] so
    RoPE becomes an operation on partition halves; the 1/sqrt(head_dim)
    score scale is folded into wq/wk on the host; the RoPE rotation sign is
    folded into the sin table ([-sin; +sin] over the two partition halves).
  - On device: qkv projection (feat-on-partition layout), RoPE fused into
    the PSUM evacuation (ACT casts PSUM->bf16, DVE does swap + 3 bf16 ops),
    causal attention computed transposed (S^T = K @ Q^T per block) so the
    PV matmul needs no transposes and the softmax denominator comes from a
    ones-matmul; softmax max-subtraction is skipped (scores ~ N(0,1));
    the attention inner loop is software-pipelined (S^T matmul+exp issued
    2 blocks ahead of the PV/denominator matmuls).
  - AllGather per (head, batch) of normalized attention outputs (4 x 0.5MB
    bf16), overlapped with remaining attention compute; each core then
    computes its 256 output columns of the wo projection.
  - Host gathers per-core (256, 4096) transposed outputs into the full
    (2, 2048, 2048) fp32 output.
"""

import sys
import numpy as np

for _p in (
    "/root/.axon_site",
    "/root/.axon_site/_ro/trn_rl_repo",
    "/root/.axon_site/_ro/pypackages",
    "/opt/trn_rl_repo",
):
    if _p not in sys.path:
        sys.path.append(_p)

import ml_dtypes

BF16 = ml_dtypes.bfloat16

B, S, DIM = 2, 2048, 2048
N_HEADS = 16
N_KV_HEADS = 4
HEAD_DIM = 128
N_CORES = 8
PE_N = 512  # moving-operand free dim per matmul


# --------------------------------------------------------------------------
# device kernel builder
# --------------------------------------------------------------------------

def build_nc(b=B, s=S):
    """Build + compile the SPMD Bass graph (identical on all 8 cores)."""
    from contextlib import ExitStack

    from concourse import bacc, mybir
    import concourse.tile as tile

    dt = mybir.dt
    f32, bf16 = dt.float32, dt.bfloat16
    rows = b * s
    KC = DIM // 128          # contraction chunks (16)
    KHALF = KC // 2
    RCB = s // PE_N          # row chunks per batch
    NKB = s // 128           # key blocks per batch
    NF = DIM // 128          # feature chunks for out-proj (16)

    nc = bacc.Bacc("TRN2", target_bir_lowering=False, debug=False,
                   num_devices=N_CORES)

    d = {}
    d["xT"] = nc.dram_tensor("xT", [DIM, rows], bf16, kind="ExternalInput")
    d["wq"] = nc.dram_tensor("wq", [DIM, 256], bf16, kind="ExternalInput")
    d["wk"] = nc.dram_tensor("wk", [DIM, 128], bf16, kind="ExternalInput")
    d["wv"] = nc.dram_tensor("wv", [DIM, 128], bf16, kind="ExternalInput")
    d["wo"] = nc.dram_tensor("wo", [DIM, 256], bf16, kind="ExternalInput")
    d["cosF"] = nc.dram_tensor("cosF", [128, s], bf16, kind="ExternalInput")
    d["sinPM"] = nc.dram_tensor("sinPM", [128, s], bf16, kind="ExternalInput")
    d["tri"] = nc.dram_tensor("tri", [128, 128], bf16, kind="ExternalInput")
    d["onesw"] = nc.dram_tensor("onesw", [128, 128], bf16, kind="ExternalInput")
    d["ident"] = nc.dram_tensor("ident", [128, 128], bf16, kind="ExternalInput")
    d["out"] = nc.dram_tensor("out", [256, rows], f32, kind="ExternalOutput")

    RCB = s // PE_N
    halves = ([(0, RCB // 2), (RCB // 2, RCB)] if RCB >= 2 else [(0, RCB)])
    hw_ = [(hi - lo) * PE_N for lo, hi in halves]
    bounce = [[[nc.dram_tensor(f"bounce{h}_{bi}_{hf}", [128, hw_[hf]], bf16)
                for hf in range(len(halves))] for bi in range(b)] for h in range(2)]
    gath = [[[nc.dram_tensor(f"gath{h}_{bi}_{hf}", [128 * N_CORES, hw_[hf]], bf16,
                             addr_space="Shared")
              for hf in range(len(halves))] for bi in range(b)] for h in range(2)]

    Exp = mybir.ActivationFunctionType.Exp

    with tile.TileContext(nc) as tc, ExitStack() as ctx:
        cpool = ctx.enter_context(tc.tile_pool(name="consts", bufs=1))
        apool = ctx.enter_context(tc.tile_pool(name="acts", bufs=1))
        tpool = ctx.enter_context(tc.tile_pool(name="tmps", bufs=4))
        epool = ctx.enter_context(tc.tile_pool(name="exps", bufs=6))
        rpool = ctx.enter_context(tc.tile_pool(name="recip", bufs=2))
        mmps = ctx.enter_context(tc.tile_pool(name="mmps", bufs=2, space="PSUM"))
        stps = ctx.enter_context(tc.tile_pool(name="stps", bufs=4, space="PSUM"))
        otps = ctx.enter_context(tc.tile_pool(name="otps", bufs=1, space="PSUM"))
        dnps = ctx.enter_context(tc.tile_pool(name="dnps", bufs=1, space="PSUM"))

        # ---- constants into SBUF
        wq_sb = cpool.tile([128, KC, 256], bf16, tag="wq")
        nc.sync.dma_start(out=wq_sb[:], in_=d["wq"].ap().rearrange("(kc p) f -> p kc f", p=128))
        wk_sb = cpool.tile([128, KC, 128], bf16, tag="wk")
        nc.sync.dma_start(out=wk_sb[:], in_=d["wk"].ap().rearrange("(kc p) f -> p kc f", p=128))
        wv_sb = cpool.tile([128, KC, 128], bf16, tag="wv")
        nc.sync.dma_start(out=wv_sb[:], in_=d["wv"].ap().rearrange("(kc p) f -> p kc f", p=128))
        wo_sb = cpool.tile([128, NF, 256], bf16, tag="wo")
        cos_sb = cpool.tile([128, s], bf16, tag="cos")
        nc.sync.dma_start(out=cos_sb[:], in_=d["cosF"][:, :])
        sin_sb = cpool.tile([128, s], bf16, tag="sin")
        nc.sync.dma_start(out=sin_sb[:], in_=d["sinPM"][:, :])
        tri_sb = cpool.tile([128, 128], bf16, tag="tri")
        ones_sb = cpool.tile([128, 128], bf16, tag="ones")
        id_sb = cpool.tile([128, 128], bf16, tag="id")

        def late_consts():
            nc.sync.dma_start(out=tri_sb[:], in_=d["tri"][:, :])
            nc.sync.dma_start(out=wo_sb[:], in_=d["wo"].ap().rearrange("(kc p) f -> p kc f", p=128))
            nc.sync.dma_start(out=ones_sb[:], in_=d["onesw"][:, :])
            nc.sync.dma_start(out=id_sb[:], in_=d["ident"][:, :])

        # ---- persistent activations
        q_sb = apool.tile([128, 2, rows], bf16, tag="q")     # qT per head (rope'd)
        kT_sb = apool.tile([128, rows], bf16, tag="k")       # kT (rope'd)
        vT_sb = apool.tile([128, rows], bf16, tag="vt")      # vT (pre-transpose)
        vn_sb = apool.tile([128, b * NKB, 128], bf16, tag="vn")  # v natural, per key-block
        ot_sb = apool.tile([128, 2, rows], bf16, tag="ot")   # normalized attn out^T

        def rope_evac(psum, dst, scol):
            """dst = rope(psum) in bf16: c1 = cast(psum); sw = swap halves;
            dst = c1*cosF + sw*sinPM (sign folded into sinPM)."""
            c1 = tpool.tile([128, PE_N], bf16, tag="c1", name="c1")
            nc.vector.tensor_copy(c1[:], psum[:])
            sw = tpool.tile([128, PE_N], bf16, tag="sw", name="sw")
            nc.vector.tensor_copy(sw[0:64, :], c1[64:128, :])
            nc.vector.tensor_copy(sw[64:128, :], c1[0:64, :])
            m1 = tpool.tile([128, PE_N], bf16, tag="m1", name="m1")
            nc.vector.tensor_mul(m1[:], c1[:], cos_sb[:, scol:scol + PE_N])
            nc.vector.tensor_mul(sw[:], sw[:], sin_sb[:, scol:scol + PE_N])
            nc.vector.tensor_add(dst, m1[:], sw[:])

        def qkv_batch(bi, xpool, late=None):
            xb = [xpool.tile([128, KHALF, s], bf16, tag="xb", name=f"xb{bi}_{i}")
                  for i in range(2)]
            for half in range(2):
                for kc in range(KHALF):
                    kcg = half * KHALF + kc
                    nc.sync.dma_start(
                        out=xb[half][:, kc, :],
                        in_=d["xT"][kcg * 128:(kcg + 1) * 128, bi * s:(bi + 1) * s])
            if late is not None:
                late()
            # mb: 0 = q head0, 1 = q head1, 2 = k, 3 = v
            for mb in range(4):
                for rcg0 in range(0, RCB, 2):
                    rcs = list(range(rcg0, min(rcg0 + 2, RCB)))
                    psums = [mmps.tile([128, PE_N], f32, tag="mm", name=f"mm{mb}_{rc}")
                             for rc in rcs]
                    for kcg in range(KC):
                        half, kc = divmod(kcg, KHALF)
                        if mb == 0:
                            w_ap = wq_sb[:, kcg, 0:128]
                        elif mb == 1:
                            w_ap = wq_sb[:, kcg, 128:256]
                        elif mb == 2:
                            w_ap = wk_sb[:, kcg, :]
                        else:
                            w_ap = wv_sb[:, kcg, :]
                        for ji, rc in enumerate(rcs):
                            nc.tensor.matmul(
                                psums[ji][:], w_ap,
                                xb[half][:, kc, rc * PE_N:(rc + 1) * PE_N],
                                start=(kcg == 0), stop=(kcg == KC - 1))
                    for ji, rc in enumerate(rcs):
                        cols = bi * s + rc * PE_N
                        scol = rc * PE_N
                        if mb <= 2:
                            dst = (q_sb[:, mb, cols:cols + PE_N] if mb < 2
                                   else kT_sb[:, cols:cols + PE_N])
                            rope_evac(psums[ji], dst, scol)
                        else:
                            nc.vector.tensor_copy(vT_sb[:, cols:cols + PE_N], psums[ji][:])
            # transpose v into natural (keys-on-partition) layout
            for kb in range(NKB):
                kbg = bi * NKB + kb
                tt = stps.tile([128, 128], bf16, tag="st", name=f"tt{kbg}")
                nc.tensor.transpose(tt[:], vT_sb[:, kbg * 128:(kbg + 1) * 128], id_sb[:])
                nc.vector.tensor_copy(vn_sb[:, kbg, :], tt[:])

        def ship(h, bi, hf):
            lo, hi = halves[hf]
            nc.sync.dma_start(
                out=bounce[h][bi][hf][:, :],
                in_=ot_sb[:, h, bi * s + lo * PE_N: bi * s + hi * PE_N])
            nc.gpsimd.collective_compute(
                "AllGather", mybir.AluOpType.bypass,
                replica_groups=[list(range(N_CORES))],
                ins=[bounce[h][bi][hf].ap().opt()],
                outs=[gath[h][bi][hf].ap().opt()])

        def attn(bi, h):
            base = bi * s
            for qc in range(RCB):
                nkb = (qc + 1) * (PE_N // 128)
                otp = otps.tile([128, PE_N], f32, tag="ot", name="otp")
                dnp = dnps.tile([128, PE_N], f32, tag="dn", name="dnp")
                qs = q_sb[:, h, base + qc * PE_N: base + (qc + 1) * PE_N]
                exps = {}

                def issue_st(kb):
                    off = max(0, (kb - 4 * qc) * 128)
                    stp = stps.tile([128, PE_N], f32, tag="st", name=f"st{kb}")
                    nc.tensor.matmul(
                        stp[:, off:], kT_sb[:, base + kb * 128: base + (kb + 1) * 128],
                        qs[:, off:], start=True, stop=True)
                    ex = epool.tile([128, PE_N], bf16, tag="ex", name=f"ex{kb}")
                    nc.scalar.activation(ex[:, off:], stp[:, off:], Exp)
                    if kb >= 4 * qc:
                        nc.vector.tensor_mul(ex[:, off:off + 128],
                                             ex[:, off:off + 128], tri_sb[:])
                    exps[kb] = (ex, off)

                def issue_pv(kb):
                    ex, off = exps.pop(kb)
                    nc.tensor.matmul(otp[:, off:], vn_sb[:, bi * NKB + kb, :],
                                     ex[:, off:],
                                     start=(kb == 0), stop=(kb == nkb - 1))
                    nc.tensor.matmul(dnp[:, off:], ones_sb[:], ex[:, off:],
                                     start=(kb == 0), stop=(kb == nkb - 1))

                DEPTH = 2
                for kb in range(nkb):
                    issue_st(kb)
                    if kb >= DEPTH:
                        issue_pv(kb - DEPTH)
                for kb in range(max(0, nkb - DEPTH), nkb):
                    issue_pv(kb)

                rc_t = rpool.tile([128, PE_N], f32, tag="rc", name="rc_t")
                nc.vector.reciprocal_approx_fast(out=rc_t[:], in_=dnp[:])
                nc.vector.tensor_mul(
                    ot_sb[:, h, base + qc * PE_N: base + (qc + 1) * PE_N],
                    otp[:], rc_t[:])
                for hf, (lo, hi) in enumerate(halves):
                    if qc == hi - 1:
                        ship(h, bi, hf)

        def outproj_batch(bi, gpool, ospool):
            # out^T[c*128:(c+1)*128, rows of batch bi] = sum_f wo[f,c] @ gath
            for hf, (lo, hi) in enumerate(halves):
                j0 = lo
                jj = list(range(lo, hi))
                nj = len(jj)
                gts = []
                for f in range(NF):
                    gt = gpool.tile([128, nj * PE_N], bf16, tag="g", name=f"g{f}")
                    src = gath[f // 8][bi][hf]
                    fl = f % 8
                    nc.sync.dma_start(out=gt[:],
                                      in_=src[fl * 128:(fl + 1) * 128, :])
                    gts.append(gt)
                # 4 concurrent accumulators (2 col-chunks x 2 row-chunks)
                psums = [[stps.tile([128, PE_N], f32, tag="st", name=f"po{c}_{j}")
                          for j in jj] for c in range(2)]
                for f in range(NF):
                    for c in range(2):
                        for ji in range(nj):
                            nc.tensor.matmul(
                                psums[c][ji][:], wo_sb[:, f, c * 128:(c + 1) * 128],
                                gts[f][:, ji * PE_N:(ji + 1) * PE_N],
                                start=(f == 0), stop=(f == NF - 1))
                for c in range(2):
                    for ji, j in enumerate(jj):
                        osb = ospool.tile([128, PE_N], f32, tag="os", name="osb")
                        nc.scalar.copy(osb[:], psums[c][ji][:])
                        nc.sync.dma_start(
                            out=d["out"][c * 128:(c + 1) * 128,
                                         bi * s + j * PE_N:
                                         bi * s + (j + 1) * PE_N],
                            in_=osb[:])

        with ExitStack() as p12:
            xpool = p12.enter_context(tc.tile_pool(name="xb", bufs=2))
            qkv_batch(0, xpool, late=late_consts)
            attn(0, 0)
            attn(0, 1)
            if b > 1:
                qkv_batch(1, xpool)
                attn(1, 0)
                attn(1, 1)

        with ExitStack() as p3:
            gpool = p3.enter_context(tc.tile_pool(name="gt", bufs=18))
            ospool = p3.enter_context(tc.tile_pool(name="os", bufs=4))
            for bi in range(b):
                outproj_batch(bi, gpool, ospool)

    nc.compile()
    return nc


# --------------------------------------------------------------------------
# host-side input prep / output assembly
# --------------------------------------------------------------------------

def prep_in_maps(x, freqs_cos, freqs_sin, wq, wk, wv, wo, b=B, s=S):
    """Shard + preprocess full fp32 inputs into 8 per-core input dicts."""
    n_heads = N_HEADS
    n_kv = N_KV_HEADS
    rows = b * s
    x = np.asarray(x, np.float32)
    xT = np.ascontiguousarray(x.reshape(rows, DIM).T).astype(BF16)

    sc = float(HEAD_DIM) ** -0.25
    perm64 = np.concatenate([np.arange(0, 128, 2), np.arange(1, 128, 2)])
    qcols = np.concatenate([h * 128 + perm64 for h in range(n_heads)])
    kcols = np.concatenate([g * 128 + perm64 for g in range(n_kv)])
    wq_p = (np.asarray(wq, np.float32) * sc)[:, qcols].astype(BF16)
    wk_p = (np.asarray(wk, np.float32) * sc)[:, kcols].astype(BF16)
    wv_p = np.asarray(wv, np.float32).astype(BF16)
    # wo rows in "gathered" order: AG0 = even heads (core order), AG1 = odd
    head_order = [2 * i for i in range(N_CORES)] + [2 * i + 1 for i in range(N_CORES)]
    wo_p = np.concatenate(
        [np.asarray(wo, np.float32)[h * 128:(h + 1) * 128, :] for h in head_order],
        axis=0).astype(BF16)

    cosT = np.asarray(freqs_cos, np.float32).T  # (64, s)
    sinT = np.asarray(freqs_sin, np.float32).T
    cosF = np.ascontiguousarray(np.concatenate([cosT, cosT], axis=0)).astype(BF16)
    sinPM = np.ascontiguousarray(np.concatenate([-sinT, sinT], axis=0)).astype(BF16)

    qi = np.arange(128)[None, :]
    ki = np.arange(128)[:, None]
    tri = (qi >= ki).astype(BF16)
    onesw = np.ones((128, 128), BF16)
    ident = np.eye(128, dtype=BF16)

    in_maps = []
    for c in range(N_CORES):
        g = c // 2
        in_maps.append({
            "xT": xT,
            "wq": np.ascontiguousarray(wq_p[:, c * 256:(c + 1) * 256]),
            "wk": np.ascontiguousarray(wk_p[:, g * 128:(g + 1) * 128]),
            "wv": np.ascontiguousarray(wv_p[:, g * 128:(g + 1) * 128]),
            "wo": np.ascontiguousarray(wo_p[:, c * 256:(c + 1) * 256]),
            "cosF": cosF,
            "sinPM": sinPM,
            "tri": tri,
            "onesw": onesw,
            "ident": ident,
        })
    return in_maps


def assemble_output(results, b=B, s=S):
    rows = b * s
    out = np.empty((rows, DIM), np.float32)
    for c in range(N_CORES):
        out[:, c * 256:(c + 1) * 256] = results[c]["out"].T
    return out.reshape(b, s, DIM)


_NC_CACHE = {}


def _get_nc(b=B, s=S):
    key = (b, s)
    if key not in _NC_CACHE:
        _NC_CACHE[key] = build_nc(b, s)
    return _NC_CACHE[key]


def run(inputs, trace=False, b=B, s=S):
    """Run the kernel; returns (output, BassKernelResults)."""
    from concourse import bass_utils
    nc = _get_nc(b, s)
    in_maps = prep_in_maps(**inputs, b=b, s=s)
    res = bass_utils.run_bass_kernel_spmd(
        nc, in_maps, core_ids=list(range(N_CORES)), trace=trace)
    return assemble_output(res.results, b, s), res


def kernel(**inputs):
    out, _ = run(inputs)
    return out
